# revision 3
# baseline (speedup 1.0000x reference)
"""LJ potential + two-level segment sum (edges -> atoms -> molecules) on 8 trn2 cores.

Strategy ("shard edges by molecule"):
  The final output is only per-molecule [1000]. On the host (as the sharding
  step) we compute each edge's molecule id m_e = idx_m[idx_i[e]], order edges
  by molecule, pad each molecule's edge run to a multiple of 128, and lay the
  stream out so that every 128-edge SBUF column is molecule-pure. Pad edges
  use vec=(1e3,0,0): r=1000 > cutoff => switch==0 => exactly zero energy.

  Each core gets a contiguous slice of the padded stream as [T, 128, 3*F]
  tiles (x/y/z planes of F=512 columns). The device computes per-edge LJ
  energy with fused custom DVE ops + ACT, then reduces each tile over the
  partition axis with a PE matmul against a one-hot column selector,
  accumulating per-(tile,column) sums into PSUM. The [T, F] column sums are
  DMA'd back; the host slices them at (host-known) molecule column boundaries
  and writes the per-molecule totals.

Per-edge math (matches reference exactly up to fp rounding):
  r2 = x^2+y^2+z^2 ;  u = 1/r2 ;  p6 = u^3 ;  q = p6^2 - p6
  b  = relu(1 - relu(2r - 4)) , r = sqrt(r2)   # == clamp(5-2r, 0, 1)
  switch = b^2*(3-2b)                          # == reference switch
  y  = q * switch ;  out_m = 0.5 * sum_{e in m} y_e   (0.5 folded into SEL)
"""

import sys

import numpy as np

if "/opt/trn_rl_repo" not in sys.path:
    sys.path.insert(0, "/opt/trn_rl_repo")

N_EDGES = 20_000_000
N_ATOMS = 1_000_000
N_MOL = 1000
N_CORES = 8
P = 128          # SBUF partitions; also the molecule padding grain (edges/column)
F = 512          # columns per tile
PAD_X = 1.0e3    # pad edge vec = (1e3, 0, 0) -> r2 = 1e6 -> switch = 0 -> y = 0

_registered_ops = {}
_compiled_cache = {}


# ---------------------------------------------------------------- custom DVE ops
def _register_custom_ops():
    """Register the fused LJ ops in concourse's custom-DVE table (idempotent)."""
    if _registered_ops:
        return _registered_ops

    from concourse import dve_ops as D
    from concourse.dve_spec import Spec, Src0, Src1, C0, lower, sq, _has_src1
    from concourse.dve_uop import DveOpSpec

    def _shas(spec):
        out = {}
        for ver in ("v3", "v4"):
            s = DveOpSpec(
                name="tmp", opcode=1, uops=lower(spec, ver=ver), rd1_en=_has_src1(spec)
            )
            out[ver] = s.sha(ver)
        return out

    def _add(name, spec):
        existing = {op.name: op for op in D.OPS}
        if name in existing:
            _registered_ops[name] = existing[name]
            return
        op = D.DveOp(name, spec, subdim=False, uops_sha=_shas(spec))
        D.OPS.append(op)
        D._SUB_OPCODE_FOR_NAME[name] = D._CUSTOM_DVE_ROW_BASE + len(D.OPS) - 1
        assert D._SUB_OPCODE_FOR_NAME[name] < 0x20
        D.CUSTOM_DVE_SPECS[name] = spec
        _registered_ops[name] = op

    _add(
        "LJ_SQSUM2",
        Spec(
            body=sq(Src0) + sq(Src1),
            reference=lambda in0, in1, s0, s1, imm2: in0 * in0 + in1 * in1,
        ),
    )
    _add(
        "LJ_SQADD",
        Spec(
            body=sq(Src0) + Src1,
            reference=lambda in0, in1, s0, s1, imm2: in0 * in0 + in1,
        ),
    )
    u3 = Src0 * Src0 * Src0
    _add(
        "LJ_Q",
        Spec(
            body=sq(u3) - u3,
            reference=lambda in0, in1, s0, s1, imm2: (
                (in0.astype(np.float32) ** 3) ** 2 - in0.astype(np.float32) ** 3
            ),
        ),
    )
    _add(
        "LJ_FINAL",
        Spec(
            body=Src0 * (sq(Src1) * (C0 - (Src1 + Src1))),
            reference=lambda in0, in1, s0, s1, imm2: in0 * (in1 * in1 * (s0 - 2.0 * in1)),
        ),
    )
    return _registered_ops


# ---------------------------------------------------------------- device kernel
def _build_kernel(T):
    """Build + compile the SPMD Bass program for T tiles per core."""
    if T in _compiled_cache:
        return _compiled_cache[T]

    import concourse.bacc as bacc
    import concourse.bass as bass
    import concourse.mybir as mybir
    import concourse.tile as tile
    from concourse.dve_ops import RECIPROCAL_APPROX_FAST

    ops = _register_custom_ops()
    f32 = mybir.dt.float32
    AF = mybir.ActivationFunctionType

    nc = bacc.Bacc("TRN2", target_bir_lowering=False, debug=False)
    v_dram = nc.dram_tensor("v", [T, P, 3 * F], f32, kind="ExternalInput")
    sel_dram = nc.dram_tensor("sel", [P, 2 * T - 1], f32, kind="ExternalInput")
    out_dram = nc.dram_tensor("colsum", [T, F], f32, kind="ExternalOutput")

    with tile.TileContext(nc) as tc:
        with (
            tc.tile_pool(name="vin", bufs=3) as vin_pool,
            tc.tile_pool(name="work", bufs=2) as work,
            tc.tile_pool(name="cst", bufs=1) as cst,
            tc.tile_pool(name="psum", bufs=1, space=bass.MemorySpace.PSUM) as psump,
        ):
            sel = cst.tile([P, 2 * T - 1], f32)
            nc.sync.dma_start(sel[:], sel_dram.ap()[:])
            bias_m4 = cst.tile([P, 1], f32)
            nc.vector.memset(bias_m4[:], -4.0)
            acc = psump.tile([T, F], f32)

            for t in range(T):
                v = vin_pool.tile([P, 3 * F], f32, tag="v")
                nc.sync.dma_start(v[:], v_dram.ap()[t])
                x, y_, z = v[:, 0:F], v[:, F : 2 * F], v[:, 2 * F : 3 * F]

                r2 = work.tile([P, F], f32, tag="r2")
                nc.vector._custom_dve(ops["LJ_SQSUM2"], out=r2[:], in0=x, in1=y_)
                nc.vector._custom_dve(ops["LJ_SQADD"], out=r2[:], in0=z, in1=r2[:])

                u = work.tile([P, F], f32, tag="u")
                nc.vector._custom_dve(RECIPROCAL_APPROX_FAST, out=u[:], in0=r2[:],
                                      s0=-0.23549792, s1=2.0017324, imm2=2.0)
                q = work.tile([P, F], f32, tag="q")
                nc.vector._custom_dve(ops["LJ_Q"], out=q[:], in0=u[:])

                r = work.tile([P, F], f32, tag="r")
                nc.scalar.activation(r[:], r2[:], AF.Sqrt)
                a = work.tile([P, F], f32, tag="a")
                nc.scalar.activation(a[:], r[:], AF.Relu, bias=bias_m4[:], scale=2.0)
                b = work.tile([P, F], f32, tag="b")
                nc.scalar.activation(b[:], a[:], AF.Relu, bias=1.0, scale=-1.0)

                yv = work.tile([P, F], f32, tag="yv")
                nc.vector._custom_dve(ops["LJ_FINAL"], out=yv[:], in0=q[:], in1=b[:],
                                      s0=3.0)

                nc.tensor.matmul(
                    acc[:],
                    sel[:, (T - 1 - t) : (T - 1 - t) + T],
                    yv[:],
                    start=(t == 0),
                    stop=(t == T - 1),
                )

            res = cst.tile([T, F], f32)
            nc.vector.tensor_copy(res[:], acc[:])
            nc.sync.dma_start(out_dram.ap()[:], res[:])

    nc.compile()
    _compiled_cache[T] = nc
    return nc


# ---------------------------------------------------------------- host wrapper
def kernel(vec_ij, positions, idx_i, idx_m):
    from concourse import bass_utils

    vec = np.ascontiguousarray(np.asarray(vec_ij, dtype=np.float32))
    idx_i = np.asarray(idx_i, dtype=np.int64)
    idx_m = np.asarray(idx_m, dtype=np.int64)
    n_edges = vec.shape[0]

    # --- sharding: order edges by molecule, pad each molecule to 128 ---
    m = idx_m[idx_i]                                   # per-edge molecule id
    order = np.argsort(m, kind="stable")
    counts = np.bincount(m, minlength=N_MOL)
    cols = (counts + P - 1) // P                       # columns per molecule
    colstart = np.zeros(N_MOL + 1, dtype=np.int64)
    np.cumsum(cols, out=colstart[1:])
    c_total = int(colstart[-1])

    T = max(1, -(-c_total // (F * N_CORES)))           # tiles per core
    c_padded = N_CORES * T * F
    e_padded = c_padded * P

    # destination slot of the k-th (sorted) edge of molecule m0: 128*colstart[m0]+k
    starts = np.zeros(N_MOL + 1, dtype=np.int64)
    np.cumsum(counts, out=starts[1:])
    m_sorted = m[order]
    rank = np.arange(n_edges, dtype=np.int64) - starts[m_sorted]
    dst = colstart[m_sorted] * P + rank

    vp = np.empty((e_padded, 3), dtype=np.float32)
    vp[:, 0] = PAD_X
    vp[:, 1] = 0.0
    vp[:, 2] = 0.0
    vp[dst] = vec[order]

    # per-core planar tiles: [T, P, 3, F]; element [t,p,c,j] = comp c of
    # edge ((core*T + t)*F + j)*P + p
    in_maps = []
    sel = np.zeros((P, 2 * T - 1), dtype=np.float32)
    sel[:, T - 1] = 0.5                                # fold WELL_DEPTH*0.5 here
    per_core = T * F * P
    for c in range(N_CORES):
        chunk = vp[c * per_core : (c + 1) * per_core]  # [T*F*P, 3]
        vc = np.ascontiguousarray(
            chunk.reshape(T, F, P, 3).transpose(0, 2, 3, 1).reshape(T, P, 3 * F)
        )
        in_maps.append({"v": vc, "sel": sel})

    nc = _build_kernel(T)
    res = bass_utils.run_bass_kernel_spmd(nc, in_maps, core_ids=list(range(N_CORES)))

    colsum = np.concatenate([res.results[c]["colsum"].ravel() for c in range(N_CORES)])
    csum = np.zeros(c_padded + 1, dtype=np.float64)
    np.cumsum(colsum, dtype=np.float64, out=csum[1:])
    y = csum[colstart[1:]] - csum[colstart[:-1]]
    return y.astype(np.float32)
